# revision 27
# baseline (speedup 1.0000x reference)
"""Bridgeout FC layer (dense_mlp) Trainium2 kernel.

out[b, o] = sum_i x[b,i] * (w[i,o] + |w[i,o]| * noise[b,i,o]) + bias[o]

Strategy (8 NeuronCores, batch-parallel):
  - Each core owns 8 of the 64 samples: its x rows and noise slices.
  - Per core the dominant traffic is its 32 MB noise slice; everything is
    structured so DMA of noise is the roofline (~358 GB/s/core).
  - Layout: contraction index i on partitions, in 8 chunks of 128. One
    (128, 8*1024) f32 tile holds a full sample's noise; one DVE
    tensor_tensor multiplies it by |w| in place; the contraction with
    x[b, :] runs on the tensor engine as float32r matmuls (1 cycle/row)
    that accumulate x@w, bias and the noise term into a single
    (8, 512)-per-half PSUM group.
"""

import os
from contextlib import ExitStack

import numpy as np

import concourse.bass as bass
import concourse.mybir as mybir
import concourse.tile as tile
from concourse.bass_utils import run_bass_kernel_spmd

F32 = mybir.dt.float32
F32R = mybir.dt.float32r
ABS = mybir.ActivationFunctionType.Abs
COPY = mybir.ActivationFunctionType.Copy

N_CORES = 8
BS, IN_F, OUT_F = 64, 1024, 1024
BL = BS // N_CORES  # samples per core
P = 128  # SBUF partitions
NCH = IN_F // P  # contraction chunks of 128
HF = 512  # psum half width (one fp32 bank)
NHALF = OUT_F // HF

# Matmul dtype: float32r streams at 1 col/cycle (vs 4 for float32) at N>=256.
MM_DT = F32R

CG = 4  # contraction chunks per noise DMA / product tile (2 MB transfers)
NG = 2  # groups per sample (CG * NG == NCH)
NOISE_BUFS = 2
PROD_BUFS = 2


def _split_multi_waits(nc: bass.Bass) -> None:
    """walrus codegen on this toolchain accepts at most ONE sync-wait per
    instruction. Tile emits joins with several waits; hoist all but the last
    onto standalone EventSemaphore instructions (what wait_ge lowers to)
    immediately before the instruction, on the same engine stream."""
    n = 0
    for func in nc.m.functions:
        for block in func.blocks:
            out = []
            changed = False
            for inst in block.instructions:
                si = inst.sync_info
                if si is not None and si.on_wait and len(si.on_wait) > 1:
                    waits = list(si.on_wait)
                    for k, w in enumerate(waits[:-1]):
                        ev = mybir.InstEventSemaphore(
                            name=f"{inst.name}-sw{k}",
                            engine=inst.engine,
                            sync_info=mybir.SyncInfo(on_wait=[w], on_update=[]),
                        )
                        nc.register_instruction(ev)
                        out.append(ev)
                        n += 1
                    inst.sync_info = mybir.SyncInfo(
                        on_wait=[waits[-1]], on_update=list(si.on_update or [])
                    )
                    changed = True
                out.append(inst)
            if changed:
                block.instructions = out


def build_bass() -> bass.Bass:
    nc = bass.Bass(trn_type="TRN2", target_bir_lowering=False, debug=False)

    x_d = nc.dram_tensor("x", [BL, IN_F], F32, kind="ExternalInput").ap()
    w_d = nc.dram_tensor("weight", [IN_F, OUT_F], F32, kind="ExternalInput").ap()
    b_d = nc.dram_tensor("bias", [OUT_F], F32, kind="ExternalInput").ap()
    n_d = nc.dram_tensor("noise", [BL, IN_F, OUT_F], F32, kind="ExternalInput").ap()
    o_d = nc.dram_tensor("out", [BL, OUT_F], F32, kind="ExternalOutput").ap()

    with tile.TileContext(nc) as tc, ExitStack() as ctx:
        const = ctx.enter_context(tc.tile_pool(name="const", bufs=1))
        psump = ctx.enter_context(tc.tile_pool(name="psum", bufs=2, space="PSUM"))

        # Full weight, laid out [p, (c o)] with i = c*128 + p.
        wq_sb = const.tile([P, NCH * OUT_F], F32)
        w_r = const.tile([P, NCH * OUT_F], MM_DT)
        w_sb = const.tile([P, NCH * OUT_F], F32)
        nc.sync.dma_start(
            w_sb[:].rearrange("p (c o) -> p c o", c=NCH),
            w_d.rearrange("(c p) o -> p c o", p=P),
        )
        nc.scalar.activation(wq_sb[:], w_sb[:], ABS)
        nc.vector.tensor_copy(w_r[:], w_sb[:])

        # x transposed: xT[p, j*NCH + c] = x[j, c*128 + p]
        xT = const.tile([P, BL * NCH], F32)
        nc.sync.dma_start(
            xT[:].rearrange("p (j c) -> p j c", c=NCH),
            x_d.rearrange("j (c p) -> p j c", p=P),
        )
        xT_r = const.tile([P, BL * NCH], MM_DT)
        nc.vector.tensor_copy(xT_r[:], xT[:])

        bias_sb = const.tile([1, OUT_F], F32)
        nc.sync.dma_start(bias_sb[:], b_d.rearrange("(u o) -> u o", u=1))
        bias_r = const.tile([1, OUT_F], MM_DT)
        nc.vector.tensor_copy(bias_r[:], bias_sb[:])
        ones_f = const.tile([1, 1], F32)
        nc.vector.memset(ones_f[:], 1.0)
        ones = const.tile([1, 1], MM_DT)
        nc.vector.tensor_copy(ones[:], ones_f[:])

        noisep = ctx.enter_context(tc.tile_pool(name="noise", bufs=NOISE_BUFS))
        prodp = ctx.enter_context(tc.tile_pool(name="prod", bufs=PROD_BUFS))
        outp = ctx.enter_context(tc.tile_pool(name="outp", bufs=2))

        # One sample at a time, in NG groups of CG contraction chunks; per
        # sample, per 512-wide half, one PSUM accumulation group holds
        # x@w + bias + the noise term.
        for j in range(BL):
            accs = [
                psump.tile([1, HF], F32, name=f"acc{j}_{h}", tag=f"acc{h}")
                for h in range(NHALF)
            ]
            for g in range(NG):
                nt = noisep.tile([P, CG * OUT_F], F32, name="nt", tag="nt")
                nc.sync.dma_start(
                    nt[:].rearrange("p (c o) -> p c o", c=CG),
                    n_d[j, g * CG * P : (g + 1) * CG * P, :].rearrange(
                        "(c p) o -> p c o", p=P
                    ),
                )
                pt = prodp.tile([P, CG * OUT_F], MM_DT, name="pt", tag="pt")
                nc.vector.tensor_mul(
                    pt[:],
                    nt[:],
                    wq_sb[:, g * CG * OUT_F : (g + 1) * CG * OUT_F],
                )
                for h in range(NHALF):
                    for cl in range(CG):
                        c = g * CG + cl
                        lhsT = xT_r[:, j * NCH + c : j * NCH + c + 1]
                        nc.tensor.matmul(
                            accs[h][:, :],
                            lhsT=lhsT,
                            rhs=w_r[
                                :, c * OUT_F + h * HF : c * OUT_F + h * HF + HF
                            ],
                            start=(c == 0),
                            stop=False,
                        )
                        nc.tensor.matmul(
                            accs[h][:, :],
                            lhsT=lhsT,
                            rhs=pt[
                                :, cl * OUT_F + h * HF : cl * OUT_F + h * HF + HF
                            ],
                            start=False,
                            stop=False,
                        )
            out_sb = outp.tile([1, OUT_F], F32, name=f"out{j}", tag="out")
            for h in range(NHALF):
                # bias via K=1 matmul closes the group
                nc.tensor.matmul(
                    accs[h][:, :],
                    lhsT=ones[:],
                    rhs=bias_r[:, h * HF : (h + 1) * HF],
                    start=False,
                    stop=True,
                )
                nc.scalar.activation(
                    out_sb[:, h * HF : (h + 1) * HF], accs[h][:, :], COPY
                )
            nc.sync.dma_start(o_d[j : j + 1, :], out_sb[:])

    _split_multi_waits(nc)
    return nc


def make_in_maps(x, weight, bias, noise):
    x = np.ascontiguousarray(x, dtype=np.float32)
    weight = np.ascontiguousarray(weight, dtype=np.float32)
    bias = np.ascontiguousarray(bias, dtype=np.float32)
    noise = np.ascontiguousarray(noise, dtype=np.float32)
    return [
        {
            "x": x[k * BL : (k + 1) * BL],
            "weight": weight,
            "bias": bias,
            "noise": np.ascontiguousarray(noise[k * BL : (k + 1) * BL]),
        }
        for k in range(N_CORES)
    ]


def kernel(**inputs) -> np.ndarray:
    nc = build_bass()
    in_maps = make_in_maps(
        inputs["x"], inputs["weight"], inputs["bias"], inputs["noise"]
    )
    res = run_bass_kernel_spmd(nc, in_maps, core_ids=list(range(N_CORES)))
    return np.concatenate(
        [res.results[k]["out"] for k in range(N_CORES)], axis=0
    ).astype(np.float32)


if __name__ == "__main__":
    rng = np.random.default_rng(0)
    x = rng.standard_normal((BS, IN_F), dtype=np.float32)
    w = rng.standard_normal((IN_F, OUT_F), dtype=np.float32) * 0.03
    b = rng.standard_normal((OUT_F,), dtype=np.float32) * 0.03
    s = (rng.random((BS, IN_F, OUT_F)) < 0.5).astype(np.float32) * 2 - 1
    out = kernel(x=x, weight=w, bias=b, noise=s)
    ref = np.einsum("bi,bio->bo", x, w[None] + np.abs(w)[None] * s) + b
    err = np.abs(out - ref).max() / np.abs(ref).max()
    print("rel err:", err)


# revision 32
# speedup vs baseline: 1.1931x; 1.1931x over previous
"""Bridgeout FC layer (dense_mlp) Trainium2 kernel.

out[b, o] = sum_i x[b,i] * (w[i,o] + |w[i,o]| * noise[b,i,o]) + bias[o]

Strategy (8 NeuronCores, batch-parallel):
  - Each core owns 8 of the 64 samples: its x rows and noise slices.
  - Per core the dominant traffic is its 32 MB noise slice; everything is
    structured so DMA of noise is the roofline (~358 GB/s/core).
  - Layout: contraction index i on partitions, in 8 chunks of 128. One
    (128, 8*1024) f32 tile holds a full sample's noise; one DVE
    tensor_tensor multiplies it by |w| in place; the contraction with
    x[b, :] runs on the tensor engine as float32r matmuls (1 cycle/row)
    that accumulate x@w, bias and the noise term into a single
    (8, 512)-per-half PSUM group.
"""

import os
from contextlib import ExitStack

import numpy as np

import concourse.bass as bass
import concourse.mybir as mybir
import concourse.tile as tile
from concourse.bass_utils import run_bass_kernel_spmd

F32 = mybir.dt.float32
F32R = mybir.dt.float32r
ABS = mybir.ActivationFunctionType.Abs
COPY = mybir.ActivationFunctionType.Copy

N_CORES = 8
BS, IN_F, OUT_F = 64, 1024, 1024
BL = BS // N_CORES  # samples per core
P = 128  # SBUF partitions
NCH = IN_F // P  # contraction chunks of 128
HF = 512  # psum half width (one fp32 bank)
NHALF = OUT_F // HF

# Matmul dtype: float32r streams at 1 col/cycle (vs 4 for float32) at N>=256.
MM_DT = F32R

CG = 4  # contraction chunks per noise DMA / product tile (2 MB transfers)
NG = 2  # groups per sample (CG * NG == NCH)
NOISE_BUFS = 4
PROD_BUFS = 3


def _split_multi_waits(nc: bass.Bass) -> None:
    """walrus codegen on this toolchain accepts at most ONE sync-wait per
    instruction. Tile emits joins with several waits; hoist all but the last
    onto standalone EventSemaphore instructions (what wait_ge lowers to)
    immediately before the instruction, on the same engine stream."""
    n = 0
    for func in nc.m.functions:
        for block in func.blocks:
            out = []
            changed = False
            for inst in block.instructions:
                si = inst.sync_info
                if si is not None and si.on_wait and len(si.on_wait) > 1:
                    waits = list(si.on_wait)
                    for k, w in enumerate(waits[:-1]):
                        ev = mybir.InstEventSemaphore(
                            name=f"{inst.name}-sw{k}",
                            engine=inst.engine,
                            sync_info=mybir.SyncInfo(on_wait=[w], on_update=[]),
                        )
                        nc.register_instruction(ev)
                        out.append(ev)
                        n += 1
                    inst.sync_info = mybir.SyncInfo(
                        on_wait=[waits[-1]], on_update=list(si.on_update or [])
                    )
                    changed = True
                out.append(inst)
            if changed:
                block.instructions = out


def build_bass() -> bass.Bass:
    nc = bass.Bass(trn_type="TRN2", target_bir_lowering=False, debug=False)

    x_d = nc.dram_tensor("x", [BL, IN_F], F32, kind="ExternalInput").ap()
    w_d = nc.dram_tensor("weight", [IN_F, OUT_F], F32, kind="ExternalInput").ap()
    b_d = nc.dram_tensor("bias", [OUT_F], F32, kind="ExternalInput").ap()
    n_d = nc.dram_tensor("noise", [BL, IN_F, OUT_F], F32, kind="ExternalInput").ap()
    o_d = nc.dram_tensor("out", [BL, OUT_F], F32, kind="ExternalOutput").ap()

    with tile.TileContext(nc) as tc, ExitStack() as ctx:
        const = ctx.enter_context(tc.tile_pool(name="const", bufs=1))
        psump = ctx.enter_context(tc.tile_pool(name="psum", bufs=3, space="PSUM"))

        # Contraction index mapping: i = p*NCH + c, so each partition's
        # slice of a chunk group is a long contiguous DRAM run (16-32 KB
        # descriptors instead of 4 KB) for both weight and noise DMAs.
        # Weight loads straight into the matmul dtype via a casting SWDGE
        # DMA; |w| for the noise multiply derives from it on the scalar
        # engine. Layout [p, (c o)] with i = p*NCH + c.
        wq_sb = const.tile([P, NCH * OUT_F], F32)
        w_r = const.tile([P, NCH * OUT_F], MM_DT)
        nc.gpsimd.dma_start(
            w_r[:].rearrange("p (c o) -> p c o", c=NCH),
            w_d.rearrange("(p c) o -> p c o", c=NCH),
        )
        nc.scalar.activation(wq_sb[:], w_r[:].bitcast(F32), ABS)

        # x transposed: xT[p, j*NCH + c] = x[j, p*NCH + c]
        xT = const.tile([P, BL * NCH], F32)
        nc.sync.dma_start(
            xT[:].rearrange("p (j c) -> p j c", c=NCH),
            x_d.rearrange("j (p c) -> p j c", c=NCH),
        )
        xT_r = const.tile([P, BL * NCH], MM_DT)
        nc.vector.tensor_copy(xT_r[:], xT[:])

        bias_sb = const.tile([1, OUT_F], F32)
        nc.sync.dma_start(bias_sb[:], b_d.rearrange("(u o) -> u o", u=1))
        bias_r = const.tile([1, OUT_F], MM_DT)
        nc.vector.tensor_copy(bias_r[:], bias_sb[:])
        ones_f = const.tile([1, 1], F32)
        nc.vector.memset(ones_f[:], 1.0)
        ones = const.tile([1, 1], MM_DT)
        nc.vector.tensor_copy(ones[:], ones_f[:])

        noisep = ctx.enter_context(tc.tile_pool(name="noise", bufs=NOISE_BUFS))
        prodp = ctx.enter_context(tc.tile_pool(name="prod", bufs=PROD_BUFS))
        outp = ctx.enter_context(tc.tile_pool(name="outp", bufs=2))

        # One sample at a time, in NG groups of CG contraction chunks; per
        # sample, per 512-wide half, one PSUM accumulation group holds
        # x@w + bias + the noise term.
        for j in range(BL):
            accs = [
                psump.tile([1, HF], F32, name=f"acc{j}_{h}", tag=f"acc{h}")
                for h in range(NHALF)
            ]
            for g in range(NG):
                nt = noisep.tile([P, CG * OUT_F], F32, name="nt", tag="nt")
                nc.sync.dma_start(
                    nt[:].rearrange("p (c o) -> p c o", c=CG),
                    n_d[j].rearrange("(p c) o -> p c o", c=NCH)[
                        :, g * CG : (g + 1) * CG, :
                    ],
                )
                pt = prodp.tile([P, CG * OUT_F], MM_DT, name="pt", tag="pt")
                nc.vector.tensor_mul(
                    pt[:],
                    nt[:],
                    wq_sb[:, g * CG * OUT_F : (g + 1) * CG * OUT_F],
                )
                for h in range(NHALF):
                    for cl in range(CG):
                        c = g * CG + cl
                        lhsT = xT_r[:, j * NCH + c : j * NCH + c + 1]
                        nc.tensor.matmul(
                            accs[h][:, :],
                            lhsT=lhsT,
                            rhs=w_r[
                                :, c * OUT_F + h * HF : c * OUT_F + h * HF + HF
                            ],
                            start=(c == 0),
                            stop=False,
                        )
                        nc.tensor.matmul(
                            accs[h][:, :],
                            lhsT=lhsT,
                            rhs=pt[
                                :, cl * OUT_F + h * HF : cl * OUT_F + h * HF + HF
                            ],
                            start=False,
                            stop=False,
                        )
            out_sb = outp.tile([1, OUT_F], F32, name=f"out{j}", tag="out")
            for h in range(NHALF):
                # bias via K=1 matmul closes the group
                nc.tensor.matmul(
                    accs[h][:, :],
                    lhsT=ones[:],
                    rhs=bias_r[:, h * HF : (h + 1) * HF],
                    start=False,
                    stop=True,
                )
                nc.scalar.activation(
                    out_sb[:, h * HF : (h + 1) * HF], accs[h][:, :], COPY
                )
            nc.sync.dma_start(o_d[j : j + 1, :], out_sb[:])

    _split_multi_waits(nc)
    return nc


def make_in_maps(x, weight, bias, noise):
    x = np.ascontiguousarray(x, dtype=np.float32)
    weight = np.ascontiguousarray(weight, dtype=np.float32)
    bias = np.ascontiguousarray(bias, dtype=np.float32)
    noise = np.ascontiguousarray(noise, dtype=np.float32)
    return [
        {
            "x": x[k * BL : (k + 1) * BL],
            "weight": weight,
            "bias": bias,
            "noise": np.ascontiguousarray(noise[k * BL : (k + 1) * BL]),
        }
        for k in range(N_CORES)
    ]


def kernel(**inputs) -> np.ndarray:
    nc = build_bass()
    in_maps = make_in_maps(
        inputs["x"], inputs["weight"], inputs["bias"], inputs["noise"]
    )
    res = run_bass_kernel_spmd(nc, in_maps, core_ids=list(range(N_CORES)))
    return np.concatenate(
        [res.results[k]["out"] for k in range(N_CORES)], axis=0
    ).astype(np.float32)


if __name__ == "__main__":
    rng = np.random.default_rng(0)
    x = rng.standard_normal((BS, IN_F), dtype=np.float32)
    w = rng.standard_normal((IN_F, OUT_F), dtype=np.float32) * 0.03
    b = rng.standard_normal((OUT_F,), dtype=np.float32) * 0.03
    s = (rng.random((BS, IN_F, OUT_F)) < 0.5).astype(np.float32) * 2 - 1
    out = kernel(x=x, weight=w, bias=b, noise=s)
    ref = np.einsum("bi,bio->bo", x, w[None] + np.abs(w)[None] * s) + b
    err = np.abs(out - ref).max() / np.abs(ref).max()
    print("rel err:", err)


# revision 34
# speedup vs baseline: 1.2642x; 1.0596x over previous
"""Bridgeout FC layer (dense_mlp) Trainium2 kernel.

out[b, o] = sum_i x[b,i] * (w[i,o] + |w[i,o]| * noise[b,i,o]) + bias[o]

Strategy (8 NeuronCores, batch-parallel):
  - Each core owns 8 of the 64 samples: its x rows and noise slices.
  - Per core the dominant traffic is its 32 MB noise slice; everything is
    structured so DMA of noise is the roofline (~358 GB/s/core).
  - Layout: contraction index i on partitions, in 8 chunks of 128. One
    (128, 8*1024) f32 tile holds a full sample's noise; one DVE
    tensor_tensor multiplies it by |w| in place; the contraction with
    x[b, :] runs on the tensor engine as float32r matmuls (1 cycle/row)
    that accumulate x@w, bias and the noise term into a single
    (8, 512)-per-half PSUM group.
"""

import os
from contextlib import ExitStack

import numpy as np

import concourse.bass as bass
import concourse.mybir as mybir
import concourse.tile as tile
from concourse.bass_utils import run_bass_kernel_spmd

F32 = mybir.dt.float32
F32R = mybir.dt.float32r
ABS = mybir.ActivationFunctionType.Abs
COPY = mybir.ActivationFunctionType.Copy

N_CORES = 8
BS, IN_F, OUT_F = 64, 1024, 1024
BL = BS // N_CORES  # samples per core
P = 128  # SBUF partitions
NCH = IN_F // P  # contraction chunks of 128
HF = 512  # psum half width (one fp32 bank)
NHALF = OUT_F // HF

# Matmul dtype: float32r streams at 1 col/cycle (vs 4 for float32) at N>=256.
MM_DT = F32R

CG = 4  # contraction chunks per noise DMA / product tile (2 MB transfers)
NG = 2  # groups per sample (CG * NG == NCH)
NOISE_BUFS = 3
PROD_BUFS = 2


def _split_multi_waits(nc: bass.Bass) -> None:
    """walrus codegen on this toolchain accepts at most ONE sync-wait per
    instruction. Tile emits joins with several waits; hoist all but the last
    onto standalone EventSemaphore instructions (what wait_ge lowers to)
    immediately before the instruction, on the same engine stream."""
    n = 0
    for func in nc.m.functions:
        for block in func.blocks:
            out = []
            changed = False
            for inst in block.instructions:
                si = inst.sync_info
                if si is not None and si.on_wait and len(si.on_wait) > 1:
                    waits = list(si.on_wait)
                    for k, w in enumerate(waits[:-1]):
                        ev = mybir.InstEventSemaphore(
                            name=f"{inst.name}-sw{k}",
                            engine=inst.engine,
                            sync_info=mybir.SyncInfo(on_wait=[w], on_update=[]),
                        )
                        nc.register_instruction(ev)
                        out.append(ev)
                        n += 1
                    inst.sync_info = mybir.SyncInfo(
                        on_wait=[waits[-1]], on_update=list(si.on_update or [])
                    )
                    changed = True
                out.append(inst)
            if changed:
                block.instructions = out


def build_bass() -> bass.Bass:
    nc = bass.Bass(trn_type="TRN2", target_bir_lowering=False, debug=False)

    x_d = nc.dram_tensor("x", [BL, IN_F], F32, kind="ExternalInput").ap()
    w_d = nc.dram_tensor("weight", [IN_F, OUT_F], F32, kind="ExternalInput").ap()
    b_d = nc.dram_tensor("bias", [OUT_F], F32, kind="ExternalInput").ap()
    n_d = nc.dram_tensor("noise", [BL, IN_F, OUT_F], F32, kind="ExternalInput").ap()
    o_d = nc.dram_tensor("out", [BL, OUT_F], F32, kind="ExternalOutput").ap()

    with tile.TileContext(nc) as tc, ExitStack() as ctx:
        const = ctx.enter_context(tc.tile_pool(name="const", bufs=1))
        psump = ctx.enter_context(tc.tile_pool(name="psum", bufs=3, space="PSUM"))

        # Contraction index mapping: i = p*NCH + c, so each partition's
        # slice of a chunk group is a long contiguous DRAM run (16-32 KB
        # descriptors instead of 4 KB) for both weight and noise DMAs.
        # Layout [p, (c o)] with i = p*NCH + c. The weight loads in two
        # halves so |w| (and the f32r copy) for group 0 is ready while the
        # second half is still in flight.
        wq_sb = const.tile([P, NCH * OUT_F], F32)
        w_r = const.tile([P, NCH * OUT_F], MM_DT)
        w_sb = const.tile([P, NCH * OUT_F], F32)
        w_src = w_d.rearrange("(p c) o -> p c o", c=NCH)
        GW = NCH // NG
        for g in range(NG):
            lo, hi = g * GW * OUT_F, (g + 1) * GW * OUT_F
            nc.sync.dma_start(
                w_sb[:, lo:hi].rearrange("p (c o) -> p c o", c=GW),
                w_src[:, g * GW : (g + 1) * GW, :],
            )
            nc.scalar.activation(wq_sb[:, lo:hi], w_sb[:, lo:hi], ABS)
            nc.vector.tensor_copy(w_r[:, lo:hi], w_sb[:, lo:hi])

        # x transposed: xT[p, j*NCH + c] = x[j, p*NCH + c]
        xT = const.tile([P, BL * NCH], F32)
        nc.sync.dma_start(
            xT[:].rearrange("p (j c) -> p j c", c=NCH),
            x_d.rearrange("j (p c) -> p j c", c=NCH),
        )
        xT_r = const.tile([P, BL * NCH], MM_DT)
        nc.vector.tensor_copy(xT_r[:], xT[:])

        bias_sb = const.tile([1, OUT_F], F32)
        nc.sync.dma_start(bias_sb[:], b_d.rearrange("(u o) -> u o", u=1))
        bias_r = const.tile([1, OUT_F], MM_DT)
        nc.vector.tensor_copy(bias_r[:], bias_sb[:])
        ones_f = const.tile([1, 1], F32)
        nc.vector.memset(ones_f[:], 1.0)
        ones = const.tile([1, 1], MM_DT)
        nc.vector.tensor_copy(ones[:], ones_f[:])

        noisep = ctx.enter_context(tc.tile_pool(name="noise", bufs=NOISE_BUFS))
        prodp = ctx.enter_context(tc.tile_pool(name="prod", bufs=PROD_BUFS))
        outp = ctx.enter_context(tc.tile_pool(name="outp", bufs=2))

        # One sample at a time, in NG groups of CG contraction chunks; per
        # sample, per 512-wide half, one PSUM accumulation group holds
        # x@w + bias + the noise term.
        for j in range(BL):
            accs = [
                psump.tile([1, HF], F32, name=f"acc{j}_{h}", tag=f"acc{h}")
                for h in range(NHALF)
            ]
            for g in range(NG):
                nt = noisep.tile([P, CG * OUT_F], F32, name="nt", tag="nt")
                nc.sync.dma_start(
                    nt[:].rearrange("p (c o) -> p c o", c=CG),
                    n_d[j].rearrange("(p c) o -> p c o", c=NCH)[
                        :, g * CG : (g + 1) * CG, :
                    ],
                )
                pt = prodp.tile([P, CG * OUT_F], MM_DT, name="pt", tag="pt")
                nc.vector.tensor_mul(
                    pt[:],
                    nt[:],
                    wq_sb[:, g * CG * OUT_F : (g + 1) * CG * OUT_F],
                )
                for h in range(NHALF):
                    for cl in range(CG):
                        c = g * CG + cl
                        lhsT = xT_r[:, j * NCH + c : j * NCH + c + 1]
                        nc.tensor.matmul(
                            accs[h][:, :],
                            lhsT=lhsT,
                            rhs=w_r[
                                :, c * OUT_F + h * HF : c * OUT_F + h * HF + HF
                            ],
                            start=(c == 0),
                            stop=False,
                        )
                        nc.tensor.matmul(
                            accs[h][:, :],
                            lhsT=lhsT,
                            rhs=pt[
                                :, cl * OUT_F + h * HF : cl * OUT_F + h * HF + HF
                            ],
                            start=False,
                            stop=False,
                        )
            out_sb = outp.tile([1, OUT_F], F32, name=f"out{j}", tag="out")
            for h in range(NHALF):
                # bias via K=1 matmul closes the group
                nc.tensor.matmul(
                    accs[h][:, :],
                    lhsT=ones[:],
                    rhs=bias_r[:, h * HF : (h + 1) * HF],
                    start=False,
                    stop=True,
                )
                nc.scalar.activation(
                    out_sb[:, h * HF : (h + 1) * HF], accs[h][:, :], COPY
                )
            nc.sync.dma_start(o_d[j : j + 1, :], out_sb[:])

    _split_multi_waits(nc)
    return nc


def make_in_maps(x, weight, bias, noise):
    x = np.ascontiguousarray(x, dtype=np.float32)
    weight = np.ascontiguousarray(weight, dtype=np.float32)
    bias = np.ascontiguousarray(bias, dtype=np.float32)
    noise = np.ascontiguousarray(noise, dtype=np.float32)
    return [
        {
            "x": x[k * BL : (k + 1) * BL],
            "weight": weight,
            "bias": bias,
            "noise": np.ascontiguousarray(noise[k * BL : (k + 1) * BL]),
        }
        for k in range(N_CORES)
    ]


def kernel(**inputs) -> np.ndarray:
    nc = build_bass()
    in_maps = make_in_maps(
        inputs["x"], inputs["weight"], inputs["bias"], inputs["noise"]
    )
    res = run_bass_kernel_spmd(nc, in_maps, core_ids=list(range(N_CORES)))
    return np.concatenate(
        [res.results[k]["out"] for k in range(N_CORES)], axis=0
    ).astype(np.float32)


if __name__ == "__main__":
    rng = np.random.default_rng(0)
    x = rng.standard_normal((BS, IN_F), dtype=np.float32)
    w = rng.standard_normal((IN_F, OUT_F), dtype=np.float32) * 0.03
    b = rng.standard_normal((OUT_F,), dtype=np.float32) * 0.03
    s = (rng.random((BS, IN_F, OUT_F)) < 0.5).astype(np.float32) * 2 - 1
    out = kernel(x=x, weight=w, bias=b, noise=s)
    ref = np.einsum("bi,bio->bo", x, w[None] + np.abs(w)[None] * s) + b
    err = np.abs(out - ref).max() / np.abs(ref).max()
    print("rel err:", err)
